# revision 42
# baseline (speedup 1.0000x reference)
"""Trainium2 Bass kernel for nn_ClipOTLoss (CLIP-style OT/Sinkhorn loss).

Computes, for full inputs features[B,D], prototypes[K,D], logits[B,K]:
    w = normalize(prototypes, axis=1)
    sims = features @ w.T / TEMPERATURE
    soft_code = sinkhorn(sims)            (3 iters, eps=0.7)
    loss = -mean_b sum_k soft_code * log_softmax(logits)

Distribution: data-parallel over B across 8 NeuronCores; prototypes
replicated; the Sinkhorn row-marginal (sum over B per prototype k)
is a 16KB AllReduce per iteration.  Per-core partial losses are summed
on the host (no final AllReduce).

Structure (v2):
  - Sinkhorn preserves diagonal scaling: Q = E * A[k] * Bb[b] with
    E = exp(sims/eps); each iteration is one PE matvec (u-direction,
    partition-reduction over b) plus one DVE free-dim reduction
    (v-direction, against a GpSimd-broadcast A row).  No E^T is ever
    built.
  - Prologue keeps the Scalar engine on one activation table
    (Square/Sqrt) instead of thrashing Square/Ln/Exp per k-tile.
  - The iteration-0 u-matvec accumulates inside the main matmul phase,
    so the first AllReduce fires right after the last exp.
  - logits are streamed exactly once, during the AllReduce gaps:
    Scalar does exp+accum (for LSE), Vector writes EL = E*logits bf16
    into the slot vacated by w^T.  The epilogue is two DVE reduce
    sweeps:  s_b = sum_k E*A,  dotraw_b = sum_k EL*A,
    loss_b = LSE_b - dotraw_b/s_b.
"""

import os
import sys

import numpy as np

sys.path.insert(0, "/opt/trn_rl_repo")

import concourse.bass as bass  # noqa: E402
import concourse.bacc as bacc  # noqa: E402
import concourse.tile as tile  # noqa: E402
import concourse.mybir as mybir  # noqa: E402
from concourse.masks import make_identity  # noqa: E402

F32 = mybir.dt.float32
BF16 = mybir.dt.bfloat16
FP8 = mybir.dt.float8e4
AF = mybir.ActivationFunctionType
ALU = mybir.AluOpType

TEMPERATURE = 0.01
EPSILON = 0.7
NUM_ITERS = 3
TINY = 1e-8

P = 128  # partitions
NSLICE = 512  # max matmul free dim (one PSUM bank of f32)


def build_nc(B_loc=1024, K=4096, D=1024, n_cores=8):
    NB = B_loc // P  # number of 128-row b-blocks per core
    NK = K // P  # number of 128-wide k-chunks
    ND = D // P  # number of 128-deep d-chunks
    SF_F = 256.0  # feature pre-scale into fp8e4 range
    SF_W = 32.0  # prototype pre-scale into fp8e4 range
    exp_scale = 1.0 / (TEMPERATURE * EPSILON) / (SF_F * SF_W)
    r_marg = 1.0 / K
    c_marg = 1.0 / (B_loc * n_cores)
    loss_scale = 1.0 / (B_loc * n_cores)
    rg = [list(range(n_cores))]
    WARM_AR = bool(int(os.environ.get('CLIP_OT_WARM_AR', '1')))
    NWARM = 110  # PE clock warmers per AllReduce gap

    nc = bacc.Bacc(None, target_bir_lowering=False, debug=False)

    feats = nc.declare_dram_parameter("features", [B_loc, D], F32, isOutput=False)
    protos = nc.declare_dram_parameter("prototypes", [K, D], F32, isOutput=False)
    logits = nc.declare_dram_parameter("logits", [B_loc, K], F32, isOutput=False)
    out_ext = nc.declare_dram_parameter("out", [1], F32, isOutput=True)

    # collective bounce buffers (internal DRAM; outputs must be Shared)
    m_in_d = [nc.dram_tensor(f"cc_m_in{i}", [K], BF16) for i in range(NUM_ITERS)]
    m_out_d = [
        nc.dram_tensor(f"cc_m_out{i}", [K], BF16, addr_space="Shared")
        for i in range(NUM_ITERS)
    ]
    w_in_d = nc.dram_tensor("cc_w_in", [8], F32)
    w_out_d = nc.dram_tensor("cc_w_out", [8], F32, addr_space="Shared")

    with tile.TileContext(nc) as tc:
        with (
            tc.tile_pool(name="single", bufs=1) as single,
            tc.tile_pool(name="big", bufs=1) as bigp,
            tc.tile_pool(name="stage", bufs=6) as stage,
            tc.tile_pool(name="wsc", bufs=6) as wscp,
            tc.tile_pool(name="psmm", bufs=2, space="PSUM") as psmm,
            tc.tile_pool(name="psmv", bufs=2, space="PSUM") as psmv,
        ):
            # ---- packed small-tensor arenas ----
            smf = single.tile([P, 256 + 128 + NK * 3 + NB * 40], F32, tag="smf")
            smb = single.tile([P, 256 + NB + 8], BF16, tag="smb")

            class _Cols:
                def __init__(self, t):
                    self.t, self.off = t, 0

                def take(self, np_, nf):
                    ap = self.t[:np_, self.off : self.off + nf]
                    self.off += nf
                    return ap

            cf, cb = _Cols(smf), _Cols(smb)

            ident_f = cf.take(P, P)
            make_identity(nc, ident_f)
            ones_f = cf.take(P, 1)
            nc.vector.memset(ones_f, 1.0)
            norm2 = cf.take(P, NK)
            sqrtn = cf.take(P, NK)
            rn = cf.take(P, NK)
            cs_fl = cf.take(P, NB * 8)  # per-eighth colsum partials
            cs0 = cf.take(P, NB)
            Bb = cf.take(P, NB)
            bt = cf.take(P, NB)
            se_fl = cf.take(P, NB * 4)  # per-quarter sum(exp(logits))
            se_s = cf.take(P, NB)
            lse = cf.take(P, NB)
            s_col = cf.take(P, NB)
            dotraw = cf.take(P, NB)
            rs = cf.take(P, NB)
            dots = cf.take(P, NB)
            losses = cf.take(P, NB)
            lcol = cf.take(P, 1)
            loss_sb = cf.take(1, 8)
            warm_src = cf.take(P, 8)
            s4 = cf.take(P, NB * 4)  # tail quarter-partials (s)
            d4 = cf.take(P, NB * 4)  # tail quarter-partials (dot)
            v4 = cf.take(P, NB * 4)  # v-sweep quarter-partials

            ident_b = cb.take(P, P)
            make_identity(nc, ident_b)
            Bb_bf = cb.take(P, NB)
            ones_b1 = cb.take(1, P)
            nc.vector.memset(ones_b1, 1.0)

            # [32, 128]-layout iteration state: x[a, b] = x[k = a*128 + b]
            m32 = [single.tile([NK, P], BF16, tag=f"m32_{i}", name=f"m32_{i}") for i in range(NUM_ITERS)]
            A32 = single.tile([NK, P], F32, tag="A32")
            A32t = single.tile([NK, P], F32, tag="A32t")
            A32bf = single.tile([NK, P], BF16, tag="A32bf")
            at_flat = single.tile([1, K], BF16, tag="atflat")
            A_bc = single.tile([P, K], BF16, tag="Abc")
            msb = single.tile([1, K], BF16, tag="msb")  # m staging row (bf16 AR payload)
            warm_sb = single.tile([1, 8], F32, tag="warmsb")

            # ---- persistent big tensors ----
            E = bigp.tile([P, NB, K], BF16, tag="E")  # E[b,k], b-major
            wn_t = bigp.tile([P, ND, K], FP8, tag="W")  # w_norm^T [d,k], fp8
            f_t = bigp.tile([P, ND, B_loc], FP8, tag="F")  # features^T [d,b], fp8

            # =========================================================
            # Warm-up AllReduce: absorbs the CC-stream startup latency
            # so the first real AllReduce triggers immediately.
            # =========================================================
            if WARM_AR:
                nc.vector.memset(warm_sb[:], 0.0)
                nc.sync.dma_start(out=w_in_d[:], in_=warm_sb[:1, :])
                nc.gpsimd.collective_compute(
                    "AllReduce", ALU.add, replica_groups=rg,
                    ins=[w_in_d[:]], outs=[w_out_d[:]],
                )

            # =========================================================
            # Prologue A: prototypes -> normalized, bf16, transposed.
            # Scalar stays on the Sqrt table (Square lives in every
            # table) -- one ACT_TABLE_LOAD for the whole prologue.
            # =========================================================
            for kt in range(NK):
                wt = stage.tile([P, D], F32, tag="stage")
                nc.sync.dma_start(out=wt[:], in_=protos[kt * P : (kt + 1) * P, :])
                ws = wscp.tile([P, D], BF16, tag="wsc")
                nc.scalar.activation(
                    out=ws[:], in_=wt[:], func=AF.Square,
                    accum_out=norm2[:, kt : kt + 1],
                )
                # sqrt(norm2/SF_W^2) = ||w||/SF_W, so rn = SF_W/||w||
                nc.scalar.activation(
                    out=sqrtn[:, kt : kt + 1], in_=norm2[:, kt : kt + 1],
                    func=AF.Sqrt, scale=1.0 / (SF_W * SF_W),
                )
                nc.vector.reciprocal(out=rn[:, kt : kt + 1], in_=sqrtn[:, kt : kt + 1])
                # scale rows by 1/||w||, cast to bf16 (overwrites Square scratch)
                nc.vector.tensor_scalar(
                    out=ws[:], in0=wt[:], scalar1=rn[:, kt : kt + 1], scalar2=None,
                    op0=ALU.mult,
                )
                # transpose the tile's 8 d-chunks into one PSUM bank, then
                # one strided copy into the wn_t column
                tp = psmm.tile([P, ND * P], BF16, tag="mm")
                for j in range(ND):
                    nc.tensor.transpose(
                        tp[:, j * P : (j + 1) * P],
                        ws[:, j * P : (j + 1) * P],
                        ident_b[:],
                    )
                if kt % 2 == 0:
                    nc.scalar.copy(
                        out=wn_t[:, :, kt * P : (kt + 1) * P],
                        in_=tp[:].rearrange("p (j b) -> p j b", j=ND),
                    )
                else:
                    nc.vector.tensor_copy(
                        out=wn_t[:, :, kt * P : (kt + 1) * P],
                        in_=tp[:].rearrange("p (j b) -> p j b", j=ND),
                    )

            # =========================================================
            # Prologue B: features -> bf16 (DVE cast), transposed [d, b]
            # =========================================================
            for c in range(NB):
                ft_in = stage.tile([P, D], F32, tag="stage")
                nc.sync.dma_start(out=ft_in[:], in_=feats[c * P : (c + 1) * P, :])
                fb = wscp.tile([P, D], BF16, tag="wsc")
                nc.vector.tensor_scalar(
                    out=fb[:], in0=ft_in[:], scalar1=SF_F, scalar2=None,
                    op0=ALU.mult,
                )
                tp = psmm.tile([P, ND * P], BF16, tag="mm")
                for j in range(ND):
                    nc.tensor.transpose(
                        tp[:, j * P : (j + 1) * P],
                        fb[:, j * P : (j + 1) * P],
                        ident_b[:],
                    )
                if c % 2 == 0:
                    nc.scalar.copy(
                        out=f_t[:, :, c * P : (c + 1) * P],
                        in_=tp[:].rearrange("p (j b) -> p j b", j=ND),
                    )
                else:
                    nc.vector.tensor_copy(
                        out=f_t[:, :, c * P : (c + 1) * P],
                        in_=tp[:].rearrange("p (j b) -> p j b", j=ND),
                    )

            # =========================================================
            # Main matmul: sims_raw = f @ wn^T, E = exp(scale*sims_raw)
            # per b-block c, per 512-col k-slice: psum [128, 512].
            # The iteration-0 u-matvec (m0[k] = sum_b E[b,k]*Bb0[b])
            # accumulates into two [1, K/2] PSUM tiles as blocks finish,
            # so the first AllReduce fires right after the last exp.
            # =========================================================
            KQ4 = K // 4  # u-matvec accumulates in [1, 1024] PSUM quarters

            def mv_quarters(pfx):
                return [
                    psmv.tile([1, KQ4], F32, tag=f"mvq{q}", bufs=1, name=f"{pfx}_{q}")
                    for q in range(3)
                ]

            def emit_mv_fused(c, qt, first, last):
                # quarters 0-2 (+ optional fused quarter-3 halves),
                # accumulated block-by-block
                for q in range(3):
                    for n in range(2):
                        o = q * KQ4 + n * NSLICE
                        nc.tensor.matmul(
                            qt[q][:1, n * NSLICE : (n + 1) * NSLICE],
                            Bb_bf[:, c : c + 1],
                            E[:, c, o : o + NSLICE],
                            start=first,
                            stop=last,
                        )
                for h, t in enumerate(qt[3:]):
                    o = 3 * KQ4 + h * NSLICE
                    nc.tensor.matmul(
                        t[:1, :],
                        Bb_bf[:, c : c + 1],
                        E[:, c, o : o + NSLICE],
                        start=first,
                        stop=last,
                    )

            def emit_mv_tail(it, qt, pfx):
                # copy quarters to the staging row (parallel engines),
                # then DMA + AR.  If quarter 3 wasn't fused (main phase,
                # where the mm PSUM ring is busy), run it here.
                nc.scalar.copy(out=msb[:1, 0:KQ4], in_=qt[0][:1, :])
                nc.vector.tensor_copy(out=msb[:1, KQ4 : 2 * KQ4], in_=qt[1][:1, :])
                nc.scalar.copy(out=msb[:1, 2 * KQ4 : 3 * KQ4], in_=qt[2][:1, :])
                if len(qt) > 3:
                    nc.vector.tensor_copy(
                        out=msb[:1, 3 * KQ4 : 3 * KQ4 + NSLICE], in_=qt[3][:1, :]
                    )
                    nc.scalar.copy(out=msb[:1, 3 * KQ4 + NSLICE :], in_=qt[4][:1, :])
                else:
                    q3 = psmv.tile([1, KQ4], F32, tag="mvq0", bufs=1, name=f"{pfx}_3")
                    for c in range(NB):
                        for n in range(2):
                            o = 3 * KQ4 + n * NSLICE
                            nc.tensor.matmul(
                                q3[:1, n * NSLICE : (n + 1) * NSLICE],
                                Bb_bf[:, c : c + 1],
                                E[:, c, o : o + NSLICE],
                                start=(c == 0),
                                stop=(c == NB - 1),
                            )
                    nc.vector.tensor_copy(out=msb[:1, 3 * KQ4 :], in_=q3[:1, :])
                nc.sync.dma_start(out=m_in_d[it][:], in_=msb[:1, :])
                nc.gpsimd.collective_compute(
                    "AllReduce", ALU.add, replica_groups=rg,
                    ins=[m_in_d[it][:]], outs=[m_out_d[it][:]],
                )

            KL = K // 4  # logits stream tile width

            def emit_lse(c, q):
                lt = stage.tile([P, KL], F32, tag="stage")
                nc.sync.dma_start(
                    out=lt[:],
                    in_=logits[c * P : (c + 1) * P, q * KL : (q + 1) * KL],
                )
                sexp = wscp.tile([P, KL], BF16, tag="wsc")
                nc.scalar.activation(
                    out=sexp[:], in_=lt[:], func=AF.Exp,
                    accum_out=se_fl[:, c * 4 + q : c * 4 + q + 1],
                )

            mv0 = mv_quarters("mv0")
            for c in range(NB):
                for e in range(K // NSLICE):
                    mm_ps = psmm.tile([P, NSLICE], F32, tag="mm")
                    for j in range(ND // 2):
                        nc.tensor.matmul(
                            mm_ps[:],
                            f_t[:, 2 * j : 2 * j + 2, c * P : (c + 1) * P],
                            wn_t[:, 2 * j : 2 * j + 2, e * NSLICE : (e + 1) * NSLICE],
                            start=(j == 0),
                            stop=(j == ND // 2 - 1),
                            perf_mode=mybir.MatmulPerfMode.DoubleRow,
                        )
                    nc.scalar.activation(
                        out=E[:, c, e * NSLICE : (e + 1) * NSLICE],
                        in_=mm_ps[:],
                        func=AF.Exp,
                        scale=exp_scale,
                        accum_out=cs_fl[:, c * 8 + e : c * 8 + e + 1],
                    )
                    if e % 2 == 1:
                        emit_lse(c, e // 2)
                # Bb0 for block c: 1 / sum_k E[b,k]
                nc.vector.tensor_reduce(
                    out=cs0[:, c : c + 1],
                    in_=cs_fl[:, c * 8 : (c + 1) * 8].rearrange("p (a q) -> p a q", a=1),
                    axis=mybir.AxisListType.X,
                    op=ALU.add,
                )
                nc.vector.reciprocal(out=Bb[:, c : c + 1], in_=cs0[:, c : c + 1])
                nc.vector.tensor_copy(out=Bb_bf[:, c : c + 1], in_=Bb[:, c : c + 1])
                if c >= 1:
                    emit_mv_fused(c - 1, mv0, first=(c - 1 == 0), last=False)
            emit_mv_fused(NB - 1, mv0, first=False, last=True)
            emit_mv_tail(0, mv0, "mv0")

            # =========================================================
            # logits stream: LSE partials (Scalar) + EL = E*logits bf16
            # (Vector) into the slot vacated by wn_t.  Split across the
            # AllReduce gaps.  EL[b,k] reuses tag "W".
            # =========================================================
            EL = bigp.tile([P, NB, K], BF16, tag="W")
            vscr = bigp.tile([P, K], BF16, tag="F")  # v-sweep dump, aliases f_t

            def emit_el(c):
                # second logits pass: EL = E*logits (DVE only; logits
                # re-DMA'd while the HBM link is otherwise idle)
                for q in range(4):
                    lt = stage.tile([P, KL], F32, tag="stage")
                    nc.sync.dma_start(
                        out=lt[:],
                        in_=logits[c * P : (c + 1) * P, q * KL : (q + 1) * KL],
                    )
                    nc.vector.tensor_tensor(
                        out=EL[:, c, q * KL : (q + 1) * KL],
                        in0=E[:, c, q * KL : (q + 1) * KL],
                        in1=lt[:],
                        op=ALU.mult,
                    )

            def emit_warmers(n):
                # dependency-free PE busy-work: keeps the clock governor
                # at full p-state through an AllReduce gap
                wp = psmm.tile([P, P], BF16, tag="mm")
                for _ in range(n):
                    nc.tensor.transpose(wp[:, :], ident_b[:], ident_b[:])

            def emit_gated_warmers(it, n):
                # warmers that depend on the AllReduce result: they run
                # right before the u-matvec, bridging the A-update gap
                wp = psmm.tile([P, NK], BF16, tag="mm")
                for _ in range(n):
                    nc.tensor.transpose(wp[:, :NK], m32[it][:, :], ident_b[:NK, :NK])

            def emit_A_update(it):
                # m arrives as [32, 128]; A update stays in that layout.
                nc.sync.dma_start(
                    out=m32[it][:], in_=m_out_d[it][:].rearrange("(a b) -> a b", a=NK)
                )
                if it < NUM_ITERS - 1:
                    emit_gated_warmers(it, 40)
                if it == 0:
                    nc.vector.tensor_scalar(
                        out=A32t[:], in0=m32[it][:], scalar1=TINY, scalar2=None,
                        op0=ALU.add,
                    )
                    nc.vector.reciprocal(out=A32[:], in_=A32t[:])
                    nc.vector.tensor_scalar(
                        out=A32[:], in0=A32[:], scalar1=r_marg, scalar2=None,
                        op0=ALU.mult,
                    )
                else:
                    nc.vector.tensor_tensor(
                        out=A32t[:], in0=A32[:], in1=m32[it][:], op=ALU.mult
                    )
                    nc.vector.tensor_scalar(
                        out=A32t[:], in0=A32t[:], scalar1=TINY, scalar2=None,
                        op0=ALU.add,
                    )
                    nc.vector.reciprocal(out=A32t[:], in_=A32t[:])
                    nc.vector.tensor_tensor(
                        out=A32[:], in0=A32[:], in1=A32t[:], op=ALU.mult
                    )
                    nc.vector.tensor_scalar(
                        out=A32[:], in0=A32[:], scalar1=r_marg, scalar2=None,
                        op0=ALU.mult,
                    )
                nc.vector.tensor_copy(out=A32bf[:], in_=A32[:])
                nc.sync.dma_start(out=at_flat[:1, :], in_=A32bf[:])
                # broadcast the A row to all partitions on the (idle) PE:
                # ones[1,128]^T @ at_flat[1,512] -> [128,512] per slice
                for n in range(K // NSLICE):
                    bc_ps = psmm.tile([P, NSLICE], F32, tag="mm")
                    nc.tensor.matmul(
                        bc_ps[:],
                        ones_b1[:1, :],
                        at_flat[:1, n * NSLICE : (n + 1) * NSLICE],
                        start=True,
                        stop=True,
                    )
                    if n % 2 == 0:
                        nc.scalar.copy(
                            out=A_bc[:, n * NSLICE : (n + 1) * NSLICE], in_=bc_ps[:]
                        )
                    else:
                        nc.vector.tensor_copy(
                            out=A_bc[:, n * NSLICE : (n + 1) * NSLICE], in_=bc_ps[:]
                        )

            # =========================================================
            # Sinkhorn iterations
            # =========================================================
            for it in range(NUM_ITERS):
                if it == 0:
                    emit_warmers(NWARM)
                    for c in range(3):
                        emit_el(c)
                emit_A_update(it)
                if it < NUM_ITERS - 1:
                    # v(c) -> Bb(c) -> u(c), pipelined per block.  The
                    # first blocks use one DVE STT each (accum = Bb*v_raw);
                    # the rest use DVE TT quarters reduced on the Scalar
                    # engine (engines balanced, u starts as block 0 lands).
                    mv = mv_quarters(f"mv{it + 1}") + [
                        psmm.tile([1, NSLICE], F32, tag="mm", name=f"mvx{it}_{h}")
                        for h in range(2)
                    ]

                    def finish_bb(c, folded):
                        if not folded:
                            # bt currently holds v_raw; fold Bb in
                            nc.vector.tensor_tensor(
                                out=bt[:, c : c + 1], in0=Bb[:, c : c + 1],
                                in1=bt[:, c : c + 1], op=ALU.mult,
                            )
                        nc.vector.tensor_scalar(
                            out=bt[:, c : c + 1], in0=bt[:, c : c + 1],
                            scalar1=TINY, scalar2=None, op0=ALU.add,
                        )
                        nc.vector.reciprocal(out=bt[:, c : c + 1], in_=bt[:, c : c + 1])
                        nc.vector.tensor_tensor(
                            out=Bb[:, c : c + 1], in0=Bb[:, c : c + 1],
                            in1=bt[:, c : c + 1], op=ALU.mult,
                        )
                        nc.vector.tensor_scalar(
                            out=Bb[:, c : c + 1], in0=Bb[:, c : c + 1],
                            scalar1=c_marg, scalar2=None, op0=ALU.mult,
                        )
                        nc.vector.tensor_copy(
                            out=Bb_bf[:, c : c + 1], in_=Bb[:, c : c + 1]
                        )
                        emit_mv_fused(c, mv, first=(c == 0), last=(c == NB - 1))

                    for c in range(3):
                        # accum = sum_k (E*Bb)*A = Bb * v_raw, in one op
                        nc.vector.scalar_tensor_tensor(
                            out=vscr[:],
                            in0=E[:, c, :],
                            scalar=Bb[:, c : c + 1],
                            in1=A_bc[:],
                            op0=ALU.mult,
                            op1=ALU.mult,
                            accum_out=bt[:, c : c + 1],
                        )
                        finish_bb(c, folded=True)
                    def reduce_v4(c):
                        nc.vector.tensor_reduce(
                            out=bt[:, c : c + 1],
                            in_=v4[:, c * 4 : (c + 1) * 4].rearrange(
                                "p (a q) -> p a q", a=1
                            ),
                            axis=mybir.AxisListType.X,
                            op=ALU.add,
                        )

                    # stagger: emit TTs of block c, then the (Scalar-fed)
                    # reduce of block c-1, so the DVE never waits on Scalar
                    for c in range(3, NB):
                        for q in range(4):
                            vq = wscp.tile([P, KQ4], BF16, tag="wsc")
                            nc.vector.tensor_tensor(
                                out=vq[:],
                                in0=E[:, c, q * KQ4 : (q + 1) * KQ4],
                                in1=A_bc[:, q * KQ4 : (q + 1) * KQ4],
                                op=ALU.mult,
                            )
                            nc.scalar.activation(
                                out=vq[:], in_=vq[:], func=AF.Copy,
                                accum_out=v4[:, c * 4 + q : c * 4 + q + 1],
                            )
                        if c > 3:
                            reduce_v4(c - 1)
                            finish_bb(c - 1, folded=False)
                    reduce_v4(NB - 1)
                    finish_bb(NB - 1, folded=False)
                    emit_mv_tail(it + 1, mv, f"mv{it + 1}")
                    if it == 0:
                        emit_warmers(NWARM)
                        for c in range(3, 6):
                            emit_el(c)
                    else:
                        for c in range(6, NB):
                            emit_el(c)

            # =========================================================
            # Final: s_b = sum_k E*A, dotraw_b = sum_k EL*A,
            # loss_b = LSE_b - dotraw_b / s_b
            # =========================================================
            # blocks 2-7: DVE TT quarters reduced on Scalar (runs both
            # engines); blocks 0-1: DVE STTs at the end (Scalar drains)
            for c in range(3, NB):
                for src, part in ((E, s4), (EL, d4)):
                    for q in range(4):
                        tq = wscp.tile([P, KQ4], BF16, tag="wsc")
                        nc.vector.tensor_tensor(
                            out=tq[:],
                            in0=src[:, c, q * KQ4 : (q + 1) * KQ4],
                            in1=A_bc[:, q * KQ4 : (q + 1) * KQ4],
                            op=ALU.mult,
                        )
                        nc.scalar.activation(
                            out=tq[:], in_=tq[:], func=AF.Copy,
                            accum_out=part[:, c * 4 + q : c * 4 + q + 1],
                        )
            for c in range(3):
                nc.vector.scalar_tensor_tensor(
                    out=vscr[:], in0=E[:, c, :], scalar=1.0, in1=A_bc[:],
                    op0=ALU.mult, op1=ALU.mult,
                    accum_out=s_col[:, c : c + 1],
                )
                nc.vector.scalar_tensor_tensor(
                    out=vscr[:], in0=EL[:, c, :], scalar=1.0, in1=A_bc[:],
                    op0=ALU.mult, op1=ALU.mult,
                    accum_out=dotraw[:, c : c + 1],
                )
            for c in range(3, NB):
                for part, dst in ((s4, s_col), (d4, dotraw)):
                    nc.vector.tensor_reduce(
                        out=dst[:, c : c + 1],
                        in_=part[:, c * 4 : (c + 1) * 4].rearrange(
                            "p (a q) -> p a q", a=1
                        ),
                        axis=mybir.AxisListType.X,
                        op=ALU.add,
                    )
            se_q = se_fl.rearrange("p (c q) -> p c q", q=4)
            nc.vector.tensor_reduce(
                out=se_s, in_=se_q, axis=mybir.AxisListType.X, op=ALU.add
            )
            nc.scalar.activation(out=lse, in_=se_s, func=AF.Ln)
            nc.vector.reciprocal(out=rs[:], in_=s_col[:])
            nc.vector.tensor_tensor(out=dots, in0=dotraw, in1=rs, op=ALU.mult)
            nc.vector.tensor_tensor(out=losses, in0=lse, in1=dots, op=ALU.subtract)
            nc.vector.tensor_reduce(
                out=lcol, in_=losses, axis=mybir.AxisListType.X, op=ALU.add
            )
            lp_ps = psmm.tile([1, NSLICE], F32, tag="mm")
            nc.tensor.matmul(
                lp_ps[:1, :1], ones_f[:, :1], lcol[:, :1], start=True, stop=True
            )
            nc.scalar.activation(
                out=loss_sb[:1, 0:1], in_=lp_ps[:1, :1], func=AF.Copy,
                scale=loss_scale,
            )
            nc.sync.dma_start(out=out_ext[:], in_=loss_sb[:1, 0:1])

    nc.compile()
    return nc


LAST_RESULT = None


def kernel(features, prototypes, logits):
    from concourse.bass_utils import run_bass_kernel_spmd

    global LAST_RESULT
    n_cores = 8
    B, D = features.shape
    K = prototypes.shape[0]
    B_loc = B // n_cores

    nc = build_nc(B_loc=B_loc, K=K, D=D, n_cores=n_cores)

    features = np.ascontiguousarray(features, dtype=np.float32)
    prototypes = np.ascontiguousarray(prototypes, dtype=np.float32)
    logits = np.ascontiguousarray(logits, dtype=np.float32)

    in_maps = [
        {
            "features": features[i * B_loc : (i + 1) * B_loc],
            "prototypes": prototypes,
            "logits": logits[i * B_loc : (i + 1) * B_loc],
        }
        for i in range(n_cores)
    ]
    res = run_bass_kernel_spmd(
        nc,
        in_maps,
        list(range(n_cores)),
        trace=bool(os.environ.get("CLIP_OT_TRACE")),
    )
    LAST_RESULT = res
    total = 0.0
    for i in range(n_cores):
        total += float(np.asarray(res.results[i]["out"], dtype=np.float64)[0])
    return np.float32(total)


# revision 43
# speedup vs baseline: 1.0648x; 1.0648x over previous
"""Trainium2 Bass kernel for nn_ClipOTLoss (CLIP-style OT/Sinkhorn loss).

Computes, for full inputs features[B,D], prototypes[K,D], logits[B,K]:
    w = normalize(prototypes, axis=1)
    sims = features @ w.T / TEMPERATURE
    soft_code = sinkhorn(sims)            (3 iters, eps=0.7)
    loss = -mean_b sum_k soft_code * log_softmax(logits)

Distribution: data-parallel over B across 8 NeuronCores; prototypes
replicated; the Sinkhorn row-marginal (sum over B per prototype k)
is a 16KB AllReduce per iteration.  Per-core partial losses are summed
on the host (no final AllReduce).

Structure (v2):
  - Sinkhorn preserves diagonal scaling: Q = E * A[k] * Bb[b] with
    E = exp(sims/eps); each iteration is one PE matvec (u-direction,
    partition-reduction over b) plus one DVE free-dim reduction
    (v-direction, against a GpSimd-broadcast A row).  No E^T is ever
    built.
  - Prologue keeps the Scalar engine on one activation table
    (Square/Sqrt) instead of thrashing Square/Ln/Exp per k-tile.
  - The iteration-0 u-matvec accumulates inside the main matmul phase,
    so the first AllReduce fires right after the last exp.
  - logits are streamed exactly once, during the AllReduce gaps:
    Scalar does exp+accum (for LSE), Vector writes EL = E*logits bf16
    into the slot vacated by w^T.  The epilogue is two DVE reduce
    sweeps:  s_b = sum_k E*A,  dotraw_b = sum_k EL*A,
    loss_b = LSE_b - dotraw_b/s_b.
"""

import os
import sys

import numpy as np

sys.path.insert(0, "/opt/trn_rl_repo")

import concourse.bass as bass  # noqa: E402
import concourse.bacc as bacc  # noqa: E402
import concourse.tile as tile  # noqa: E402
import concourse.mybir as mybir  # noqa: E402
from concourse.masks import make_identity  # noqa: E402

F32 = mybir.dt.float32
BF16 = mybir.dt.bfloat16
FP8 = mybir.dt.float8e4
AF = mybir.ActivationFunctionType
ALU = mybir.AluOpType

TEMPERATURE = 0.01
EPSILON = 0.7
NUM_ITERS = 3
TINY = 1e-8

P = 128  # partitions
NSLICE = 512  # max matmul free dim (one PSUM bank of f32)


def build_nc(B_loc=1024, K=4096, D=1024, n_cores=8):
    NB = B_loc // P  # number of 128-row b-blocks per core
    NK = K // P  # number of 128-wide k-chunks
    ND = D // P  # number of 128-deep d-chunks
    SF_F = 256.0  # feature pre-scale into fp8e4 range
    SF_W = 32.0  # prototype pre-scale into fp8e4 range
    exp_scale = 1.0 / (TEMPERATURE * EPSILON) / (SF_F * SF_W)
    r_marg = 1.0 / K
    c_marg = 1.0 / (B_loc * n_cores)
    loss_scale = 1.0 / (B_loc * n_cores)
    rg = [list(range(n_cores))]
    WARM_AR = bool(int(os.environ.get('CLIP_OT_WARM_AR', '1')))
    NWARM = 110  # PE clock warmers per AllReduce gap

    nc = bacc.Bacc(None, target_bir_lowering=False, debug=False)

    feats = nc.declare_dram_parameter("features", [B_loc, D], F32, isOutput=False)
    protos = nc.declare_dram_parameter("prototypes", [K, D], F32, isOutput=False)
    logits = nc.declare_dram_parameter("logits", [B_loc, K], F32, isOutput=False)
    out_ext = nc.declare_dram_parameter("out", [1], F32, isOutput=True)

    # collective bounce buffers (internal DRAM; outputs must be Shared)
    m_in_d = [nc.dram_tensor(f"cc_m_in{i}", [K], BF16) for i in range(NUM_ITERS)]
    m_out_d = [
        nc.dram_tensor(f"cc_m_out{i}", [K], BF16, addr_space="Shared")
        for i in range(NUM_ITERS)
    ]
    w_in_d = nc.dram_tensor("cc_w_in", [8], F32)
    w_out_d = nc.dram_tensor("cc_w_out", [8], F32, addr_space="Shared")

    with tile.TileContext(nc) as tc:
        with (
            tc.tile_pool(name="single", bufs=1) as single,
            tc.tile_pool(name="big", bufs=1) as bigp,
            tc.tile_pool(name="stage", bufs=6) as stage,
            tc.tile_pool(name="wsc", bufs=6) as wscp,
            tc.tile_pool(name="psmm", bufs=2, space="PSUM") as psmm,
            tc.tile_pool(name="psmv", bufs=2, space="PSUM") as psmv,
        ):
            # ---- packed small-tensor arenas ----
            smf = single.tile([P, 256 + 128 + NK * 3 + NB * 40], F32, tag="smf")
            smb = single.tile([P, 256 + NB + 8], BF16, tag="smb")

            class _Cols:
                def __init__(self, t):
                    self.t, self.off = t, 0

                def take(self, np_, nf):
                    ap = self.t[:np_, self.off : self.off + nf]
                    self.off += nf
                    return ap

            cf, cb = _Cols(smf), _Cols(smb)

            ident_f = cf.take(P, P)
            make_identity(nc, ident_f)
            ones_f = cf.take(P, 1)
            nc.vector.memset(ones_f, 1.0)
            norm2 = cf.take(P, NK)
            sqrtn = cf.take(P, NK)
            rn = cf.take(P, NK)
            cs_fl = cf.take(P, NB * 8)  # per-eighth colsum partials
            cs0 = cf.take(P, NB)
            Bb = cf.take(P, NB)
            bt = cf.take(P, NB)
            se_fl = cf.take(P, NB * 4)  # per-quarter sum(exp(logits))
            se_s = cf.take(P, NB)
            lse = cf.take(P, NB)
            s_col = cf.take(P, NB)
            dotraw = cf.take(P, NB)
            rs = cf.take(P, NB)
            dots = cf.take(P, NB)
            losses = cf.take(P, NB)
            lcol = cf.take(P, 1)
            loss_sb = cf.take(1, 8)
            warm_src = cf.take(P, 8)
            s4 = cf.take(P, NB * 4)  # tail quarter-partials (s)
            d4 = cf.take(P, NB * 4)  # tail quarter-partials (dot)
            v4 = cf.take(P, NB * 4)  # v-sweep quarter-partials

            ident_b = cb.take(P, P)
            make_identity(nc, ident_b)
            Bb_bf = cb.take(P, NB)
            ones_b1 = cb.take(1, P)
            nc.vector.memset(ones_b1, 1.0)

            # [32, 128]-layout iteration state: x[a, b] = x[k = a*128 + b]
            m32 = [single.tile([NK, P], BF16, tag=f"m32_{i}", name=f"m32_{i}") for i in range(NUM_ITERS)]
            A32 = single.tile([NK, P], F32, tag="A32")
            A32t = single.tile([NK, P], F32, tag="A32t")
            A32bf = single.tile([NK, P], BF16, tag="A32bf")
            at_flat = single.tile([1, K], BF16, tag="atflat")
            A_bc = single.tile([P, K], BF16, tag="Abc")
            msb = single.tile([1, K], BF16, tag="msb")  # m staging row (bf16 AR payload)
            warm_sb = single.tile([1, 8], F32, tag="warmsb")

            # ---- persistent big tensors ----
            E = bigp.tile([P, NB, K], BF16, tag="E")  # E[b,k], b-major
            wn_t = bigp.tile([P, ND, K], FP8, tag="W")  # w_norm^T [d,k], fp8
            f_t = bigp.tile([P, ND, B_loc], FP8, tag="F")  # features^T [d,b], fp8

            # =========================================================
            # Warm-up AllReduce: absorbs the CC-stream startup latency
            # so the first real AllReduce triggers immediately.
            # =========================================================
            if WARM_AR:
                nc.vector.memset(warm_sb[:], 0.0)
                nc.sync.dma_start(out=w_in_d[:], in_=warm_sb[:1, :])
                nc.gpsimd.collective_compute(
                    "AllReduce", ALU.add, replica_groups=rg,
                    ins=[w_in_d[:]], outs=[w_out_d[:]],
                )

            # =========================================================
            # Prologue A: prototypes -> normalized, bf16, transposed.
            # Scalar stays on the Sqrt table (Square lives in every
            # table) -- one ACT_TABLE_LOAD for the whole prologue.
            # =========================================================
            for kt in range(NK):
                wt = stage.tile([P, D], F32, tag="stage")
                nc.sync.dma_start(out=wt[:], in_=protos[kt * P : (kt + 1) * P, :])
                ws = wscp.tile([P, D], BF16, tag="wsc")
                nc.scalar.activation(
                    out=ws[:], in_=wt[:], func=AF.Square,
                    accum_out=norm2[:, kt : kt + 1],
                )
                # sqrt(norm2/SF_W^2) = ||w||/SF_W, so rn = SF_W/||w||
                nc.scalar.activation(
                    out=sqrtn[:, kt : kt + 1], in_=norm2[:, kt : kt + 1],
                    func=AF.Sqrt, scale=1.0 / (SF_W * SF_W),
                )
                nc.vector.reciprocal(out=rn[:, kt : kt + 1], in_=sqrtn[:, kt : kt + 1])
                # scale rows by 1/||w||, cast to bf16 (overwrites Square scratch)
                nc.vector.tensor_scalar(
                    out=ws[:], in0=wt[:], scalar1=rn[:, kt : kt + 1], scalar2=None,
                    op0=ALU.mult,
                )
                # transpose the tile's 8 d-chunks into one PSUM bank, then
                # one strided copy into the wn_t column
                tp = psmm.tile([P, ND * P], BF16, tag="mm")
                for j in range(ND):
                    nc.tensor.transpose(
                        tp[:, j * P : (j + 1) * P],
                        ws[:, j * P : (j + 1) * P],
                        ident_b[:],
                    )
                if kt % 2 == 0:
                    nc.scalar.copy(
                        out=wn_t[:, :, kt * P : (kt + 1) * P],
                        in_=tp[:].rearrange("p (j b) -> p j b", j=ND),
                    )
                else:
                    nc.vector.tensor_copy(
                        out=wn_t[:, :, kt * P : (kt + 1) * P],
                        in_=tp[:].rearrange("p (j b) -> p j b", j=ND),
                    )

            # =========================================================
            # Prologue B: features -> bf16 (DVE cast), transposed [d, b]
            # =========================================================
            for c in range(NB):
                ft_in = stage.tile([P, D], F32, tag="stage")
                nc.sync.dma_start(out=ft_in[:], in_=feats[c * P : (c + 1) * P, :])
                fb = wscp.tile([P, D], BF16, tag="wsc")
                nc.vector.tensor_scalar(
                    out=fb[:], in0=ft_in[:], scalar1=SF_F, scalar2=None,
                    op0=ALU.mult,
                )
                tp = psmm.tile([P, ND * P], BF16, tag="mm")
                for j in range(ND):
                    nc.tensor.transpose(
                        tp[:, j * P : (j + 1) * P],
                        fb[:, j * P : (j + 1) * P],
                        ident_b[:],
                    )
                if c % 2 == 0:
                    nc.scalar.copy(
                        out=f_t[:, :, c * P : (c + 1) * P],
                        in_=tp[:].rearrange("p (j b) -> p j b", j=ND),
                    )
                else:
                    nc.vector.tensor_copy(
                        out=f_t[:, :, c * P : (c + 1) * P],
                        in_=tp[:].rearrange("p (j b) -> p j b", j=ND),
                    )

            # =========================================================
            # Main matmul: sims_raw = f @ wn^T, E = exp(scale*sims_raw)
            # per b-block c, per 512-col k-slice: psum [128, 512].
            # The iteration-0 u-matvec (m0[k] = sum_b E[b,k]*Bb0[b])
            # accumulates into two [1, K/2] PSUM tiles as blocks finish,
            # so the first AllReduce fires right after the last exp.
            # =========================================================
            KQ4 = K // 4  # u-matvec accumulates in [1, 1024] PSUM quarters

            def mv_quarters(pfx):
                return [
                    psmv.tile([1, KQ4], F32, tag=f"mvq{q}", bufs=1, name=f"{pfx}_{q}")
                    for q in range(3)
                ]

            def emit_mv_fused(c, qt, first, last):
                # quarters 0-2 (+ optional fused quarter-3 halves),
                # accumulated block-by-block
                for q in range(3):
                    for n in range(2):
                        o = q * KQ4 + n * NSLICE
                        nc.tensor.matmul(
                            qt[q][:1, n * NSLICE : (n + 1) * NSLICE],
                            Bb_bf[:, c : c + 1],
                            E[:, c, o : o + NSLICE],
                            start=first,
                            stop=last,
                        )
                for h, t in enumerate(qt[3:]):
                    o = 3 * KQ4 + h * NSLICE
                    nc.tensor.matmul(
                        t[:1, :],
                        Bb_bf[:, c : c + 1],
                        E[:, c, o : o + NSLICE],
                        start=first,
                        stop=last,
                    )

            def emit_mv_tail(it, qt, pfx):
                # copy quarters to the staging row (parallel engines),
                # then DMA + AR.  If quarter 3 wasn't fused (main phase,
                # where the mm PSUM ring is busy), run it here.
                nc.scalar.copy(out=msb[:1, 0:KQ4], in_=qt[0][:1, :])
                nc.vector.tensor_copy(out=msb[:1, KQ4 : 2 * KQ4], in_=qt[1][:1, :])
                nc.scalar.copy(out=msb[:1, 2 * KQ4 : 3 * KQ4], in_=qt[2][:1, :])
                if len(qt) > 3:
                    nc.vector.tensor_copy(
                        out=msb[:1, 3 * KQ4 : 3 * KQ4 + NSLICE], in_=qt[3][:1, :]
                    )
                    nc.scalar.copy(out=msb[:1, 3 * KQ4 + NSLICE :], in_=qt[4][:1, :])
                else:
                    q3 = psmv.tile([1, KQ4], F32, tag="mvq0", bufs=1, name=f"{pfx}_3")
                    for c in range(NB):
                        for n in range(2):
                            o = 3 * KQ4 + n * NSLICE
                            nc.tensor.matmul(
                                q3[:1, n * NSLICE : (n + 1) * NSLICE],
                                Bb_bf[:, c : c + 1],
                                E[:, c, o : o + NSLICE],
                                start=(c == 0),
                                stop=(c == NB - 1),
                            )
                    nc.vector.tensor_copy(out=msb[:1, 3 * KQ4 :], in_=q3[:1, :])
                nc.sync.dma_start(out=m_in_d[it][:], in_=msb[:1, :])
                nc.gpsimd.collective_compute(
                    "AllReduce", ALU.add, replica_groups=rg,
                    ins=[m_in_d[it][:]], outs=[m_out_d[it][:]],
                )

            KL = K // 4  # logits stream tile width

            def emit_lse(c, q):
                lt = stage.tile([P, KL], F32, tag="stage")
                nc.sync.dma_start(
                    out=lt[:],
                    in_=logits[c * P : (c + 1) * P, q * KL : (q + 1) * KL],
                )
                sexp = wscp.tile([P, KL], BF16, tag="wsc")
                nc.scalar.activation(
                    out=sexp[:], in_=lt[:], func=AF.Exp,
                    accum_out=se_fl[:, c * 4 + q : c * 4 + q + 1],
                )

            mv0 = mv_quarters("mv0")
            for c in range(NB):
                for e in range(K // NSLICE):
                    mm_ps = psmm.tile([P, NSLICE], F32, tag="mm")
                    for j in range(ND // 2):
                        nc.tensor.matmul(
                            mm_ps[:],
                            f_t[:, 2 * j : 2 * j + 2, c * P : (c + 1) * P],
                            wn_t[:, 2 * j : 2 * j + 2, e * NSLICE : (e + 1) * NSLICE],
                            start=(j == 0),
                            stop=(j == ND // 2 - 1),
                            perf_mode=mybir.MatmulPerfMode.DoubleRow,
                        )
                    nc.scalar.activation(
                        out=E[:, c, e * NSLICE : (e + 1) * NSLICE],
                        in_=mm_ps[:],
                        func=AF.Exp,
                        scale=exp_scale,
                        accum_out=cs_fl[:, c * 8 + e : c * 8 + e + 1],
                    )
                    if e % 2 == 1:
                        emit_lse(c, e // 2)
                # Bb0 for block c: 1 / sum_k E[b,k]
                nc.vector.tensor_reduce(
                    out=cs0[:, c : c + 1],
                    in_=cs_fl[:, c * 8 : (c + 1) * 8].rearrange("p (a q) -> p a q", a=1),
                    axis=mybir.AxisListType.X,
                    op=ALU.add,
                )
                nc.vector.reciprocal(out=Bb[:, c : c + 1], in_=cs0[:, c : c + 1])
                nc.vector.tensor_copy(out=Bb_bf[:, c : c + 1], in_=Bb[:, c : c + 1])
                if c >= 1:
                    emit_mv_fused(c - 1, mv0, first=(c - 1 == 0), last=False)
            emit_mv_fused(NB - 1, mv0, first=False, last=True)
            emit_mv_tail(0, mv0, "mv0")

            # =========================================================
            # logits stream: LSE partials (Scalar) + EL = E*logits bf16
            # (Vector) into the slot vacated by wn_t.  Split across the
            # AllReduce gaps.  EL[b,k] reuses tag "W".
            # =========================================================
            EL = bigp.tile([P, NB, K], BF16, tag="W")
            vscr = bigp.tile([P, K], BF16, tag="F")  # v-sweep dump, aliases f_t

            def emit_el(c):
                # second logits pass: EL = E*logits (DVE only; logits
                # re-DMA'd while the HBM link is otherwise idle)
                for q in range(4):
                    lt = stage.tile([P, KL], F32, tag="stage")
                    nc.sync.dma_start(
                        out=lt[:],
                        in_=logits[c * P : (c + 1) * P, q * KL : (q + 1) * KL],
                    )
                    nc.vector.tensor_tensor(
                        out=EL[:, c, q * KL : (q + 1) * KL],
                        in0=E[:, c, q * KL : (q + 1) * KL],
                        in1=lt[:],
                        op=ALU.mult,
                    )

            def emit_warmers(n):
                # dependency-free PE busy-work: keeps the clock governor
                # at full p-state through an AllReduce gap
                wp = psmm.tile([P, P], BF16, tag="mm")
                for _ in range(n):
                    nc.tensor.transpose(wp[:, :], ident_b[:], ident_b[:])

            def emit_gated_warmers(it, n):
                # warmers that depend on the AllReduce result: they run
                # right before the u-matvec, bridging the A-update gap
                wp = psmm.tile([P, NK], BF16, tag="mm")
                for _ in range(n):
                    nc.tensor.transpose(wp[:, :NK], m32[it][:, :], ident_b[:NK, :NK])

            def emit_A_update(it):
                # m arrives as [32, 128]; A update stays in that layout.
                nc.sync.dma_start(
                    out=m32[it][:], in_=m_out_d[it][:].rearrange("(a b) -> a b", a=NK)
                )
                if it < NUM_ITERS - 1:
                    emit_gated_warmers(it, 40)
                if it == 0:
                    nc.vector.tensor_scalar(
                        out=A32t[:], in0=m32[it][:], scalar1=TINY, scalar2=None,
                        op0=ALU.add,
                    )
                    nc.vector.reciprocal(out=A32[:], in_=A32t[:])
                    nc.vector.tensor_scalar(
                        out=A32[:], in0=A32[:], scalar1=r_marg, scalar2=None,
                        op0=ALU.mult,
                    )
                else:
                    nc.vector.tensor_tensor(
                        out=A32t[:], in0=A32[:], in1=m32[it][:], op=ALU.mult
                    )
                    nc.vector.tensor_scalar(
                        out=A32t[:], in0=A32t[:], scalar1=TINY, scalar2=None,
                        op0=ALU.add,
                    )
                    nc.vector.reciprocal(out=A32t[:], in_=A32t[:])
                    nc.vector.tensor_tensor(
                        out=A32[:], in0=A32[:], in1=A32t[:], op=ALU.mult
                    )
                    nc.vector.tensor_scalar(
                        out=A32[:], in0=A32[:], scalar1=r_marg, scalar2=None,
                        op0=ALU.mult,
                    )
                nc.vector.tensor_copy(out=A32bf[:], in_=A32[:])
                nc.sync.dma_start(out=at_flat[:1, :], in_=A32bf[:])
                # broadcast the A row to all partitions on the (idle) PE:
                # ones[1,128]^T @ at_flat[1,512] -> [128,512] per slice
                for n in range(K // NSLICE):
                    bc_ps = psmm.tile([P, NSLICE], F32, tag="mm")
                    nc.tensor.matmul(
                        bc_ps[:],
                        ones_b1[:1, :],
                        at_flat[:1, n * NSLICE : (n + 1) * NSLICE],
                        start=True,
                        stop=True,
                    )
                    if n % 2 == 0:
                        nc.scalar.copy(
                            out=A_bc[:, n * NSLICE : (n + 1) * NSLICE], in_=bc_ps[:]
                        )
                    else:
                        nc.vector.tensor_copy(
                            out=A_bc[:, n * NSLICE : (n + 1) * NSLICE], in_=bc_ps[:]
                        )

            # =========================================================
            # Sinkhorn iterations
            # =========================================================
            for it in range(NUM_ITERS):
                if it == 0:
                    emit_warmers(NWARM)
                    for c in range(3):
                        emit_el(c)
                emit_A_update(it)
                if it < NUM_ITERS - 1:
                    # v(c) -> Bb(c) -> u(c), pipelined per block.  The
                    # first blocks use one DVE STT each (accum = Bb*v_raw);
                    # the rest use DVE TT quarters reduced on the Scalar
                    # engine (engines balanced, u starts as block 0 lands).
                    mv = mv_quarters(f"mv{it + 1}") + [
                        psmm.tile([1, NSLICE], F32, tag="mm", name=f"mvx{it}_{h}")
                        for h in range(2)
                    ]

                    def finish_bb(c, folded):
                        if not folded:
                            # bt currently holds v_raw; fold Bb in
                            nc.vector.tensor_tensor(
                                out=bt[:, c : c + 1], in0=Bb[:, c : c + 1],
                                in1=bt[:, c : c + 1], op=ALU.mult,
                            )
                        nc.vector.tensor_scalar(
                            out=bt[:, c : c + 1], in0=bt[:, c : c + 1],
                            scalar1=TINY, scalar2=None, op0=ALU.add,
                        )
                        nc.vector.reciprocal(out=bt[:, c : c + 1], in_=bt[:, c : c + 1])
                        nc.vector.tensor_tensor(
                            out=Bb[:, c : c + 1], in0=Bb[:, c : c + 1],
                            in1=bt[:, c : c + 1], op=ALU.mult,
                        )
                        nc.vector.tensor_scalar(
                            out=Bb[:, c : c + 1], in0=Bb[:, c : c + 1],
                            scalar1=c_marg, scalar2=None, op0=ALU.mult,
                        )
                        nc.vector.tensor_copy(
                            out=Bb_bf[:, c : c + 1], in_=Bb[:, c : c + 1]
                        )
                        emit_mv_fused(c, mv, first=(c == 0), last=(c == NB - 1))

                    for c in range(3):
                        # accum = sum_k (E*Bb)*A = Bb * v_raw, in one op
                        nc.vector.scalar_tensor_tensor(
                            out=vscr[:],
                            in0=E[:, c, :],
                            scalar=Bb[:, c : c + 1],
                            in1=A_bc[:],
                            op0=ALU.mult,
                            op1=ALU.mult,
                            accum_out=bt[:, c : c + 1],
                        )
                        finish_bb(c, folded=True)
                    def reduce_v4(c):
                        nc.vector.tensor_reduce(
                            out=bt[:, c : c + 1],
                            in_=v4[:, c * 4 : (c + 1) * 4].rearrange(
                                "p (a q) -> p a q", a=1
                            ),
                            axis=mybir.AxisListType.X,
                            op=ALU.add,
                        )

                    # stagger: emit TTs of block c, then the (Scalar-fed)
                    # reduce of block c-1, so the DVE never waits on Scalar
                    for c in range(3, NB):
                        for q in range(4):
                            vq = wscp.tile([P, KQ4], BF16, tag="wsc")
                            nc.vector.tensor_tensor(
                                out=vq[:],
                                in0=E[:, c, q * KQ4 : (q + 1) * KQ4],
                                in1=A_bc[:, q * KQ4 : (q + 1) * KQ4],
                                op=ALU.mult,
                            )
                            nc.scalar.activation(
                                out=vq[:], in_=vq[:], func=AF.Copy,
                                accum_out=v4[:, c * 4 + q : c * 4 + q + 1],
                            )
                        if c > 3:
                            reduce_v4(c - 1)
                            finish_bb(c - 1, folded=False)
                    reduce_v4(NB - 1)
                    finish_bb(NB - 1, folded=False)
                    emit_mv_tail(it + 1, mv, f"mv{it + 1}")
                    if it == 0:
                        emit_warmers(55)
                        for c in range(3, 6):
                            emit_el(c)
                    else:
                        for c in range(6, NB):
                            emit_el(c)

            # =========================================================
            # Final: s_b = sum_k E*A, dotraw_b = sum_k EL*A,
            # loss_b = LSE_b - dotraw_b / s_b
            # =========================================================
            # blocks 2-7: DVE TT quarters reduced on Scalar (runs both
            # engines); blocks 0-1: DVE STTs at the end (Scalar drains)
            for c in range(3, NB):
                for src, part in ((E, s4), (EL, d4)):
                    for q in range(4):
                        tq = wscp.tile([P, KQ4], BF16, tag="wsc")
                        nc.vector.tensor_tensor(
                            out=tq[:],
                            in0=src[:, c, q * KQ4 : (q + 1) * KQ4],
                            in1=A_bc[:, q * KQ4 : (q + 1) * KQ4],
                            op=ALU.mult,
                        )
                        nc.scalar.activation(
                            out=tq[:], in_=tq[:], func=AF.Copy,
                            accum_out=part[:, c * 4 + q : c * 4 + q + 1],
                        )
            for c in range(3):
                nc.vector.scalar_tensor_tensor(
                    out=vscr[:], in0=E[:, c, :], scalar=1.0, in1=A_bc[:],
                    op0=ALU.mult, op1=ALU.mult,
                    accum_out=s_col[:, c : c + 1],
                )
                nc.vector.scalar_tensor_tensor(
                    out=vscr[:], in0=EL[:, c, :], scalar=1.0, in1=A_bc[:],
                    op0=ALU.mult, op1=ALU.mult,
                    accum_out=dotraw[:, c : c + 1],
                )
            for c in range(3, NB):
                for part, dst in ((s4, s_col), (d4, dotraw)):
                    nc.vector.tensor_reduce(
                        out=dst[:, c : c + 1],
                        in_=part[:, c * 4 : (c + 1) * 4].rearrange(
                            "p (a q) -> p a q", a=1
                        ),
                        axis=mybir.AxisListType.X,
                        op=ALU.add,
                    )
            se_q = se_fl.rearrange("p (c q) -> p c q", q=4)
            nc.vector.tensor_reduce(
                out=se_s, in_=se_q, axis=mybir.AxisListType.X, op=ALU.add
            )
            nc.scalar.activation(out=lse, in_=se_s, func=AF.Ln)
            nc.vector.reciprocal(out=rs[:], in_=s_col[:])
            nc.vector.tensor_tensor(out=dots, in0=dotraw, in1=rs, op=ALU.mult)
            nc.vector.tensor_tensor(out=losses, in0=lse, in1=dots, op=ALU.subtract)
            nc.vector.tensor_reduce(
                out=lcol, in_=losses, axis=mybir.AxisListType.X, op=ALU.add
            )
            lp_ps = psmm.tile([1, NSLICE], F32, tag="mm")
            nc.tensor.matmul(
                lp_ps[:1, :1], ones_f[:, :1], lcol[:, :1], start=True, stop=True
            )
            nc.scalar.activation(
                out=loss_sb[:1, 0:1], in_=lp_ps[:1, :1], func=AF.Copy,
                scale=loss_scale,
            )
            nc.sync.dma_start(out=out_ext[:], in_=loss_sb[:1, 0:1])

    nc.compile()
    return nc


LAST_RESULT = None


def kernel(features, prototypes, logits):
    from concourse.bass_utils import run_bass_kernel_spmd

    global LAST_RESULT
    n_cores = 8
    B, D = features.shape
    K = prototypes.shape[0]
    B_loc = B // n_cores

    nc = build_nc(B_loc=B_loc, K=K, D=D, n_cores=n_cores)

    features = np.ascontiguousarray(features, dtype=np.float32)
    prototypes = np.ascontiguousarray(prototypes, dtype=np.float32)
    logits = np.ascontiguousarray(logits, dtype=np.float32)

    in_maps = [
        {
            "features": features[i * B_loc : (i + 1) * B_loc],
            "prototypes": prototypes,
            "logits": logits[i * B_loc : (i + 1) * B_loc],
        }
        for i in range(n_cores)
    ]
    res = run_bass_kernel_spmd(
        nc,
        in_maps,
        list(range(n_cores)),
        trace=bool(os.environ.get("CLIP_OT_TRACE")),
    )
    LAST_RESULT = res
    total = 0.0
    for i in range(n_cores):
        total += float(np.asarray(res.results[i]["out"], dtype=np.float64)[0])
    return np.float32(total)
